# revision 32
# baseline (speedup 1.0000x reference)
"""Multi-head self-attention (B=8, S=1024, D=768, H=12, dh=64) on 8 trn2 cores.

Sharding: data-parallel over batch — core b computes batch element b entirely
(Q/K/V projections + per-head softmax(QK^T/sqrt(dh))V), no collectives.

Design (v8 — quadrant QK + rebalanced softmax engines; 197us, from 216us v5):
  The kernel is a 48-step pipeline over (head-pair, k-block): QK^T scores ->
  exp (ACT exact / DVE Schraudolph in parallel) -> AV, with projections and
  output stores interleaved as PE filler. Steady-state period ~1.86us/step is
  set by the serial A-side chain [QK (430ns warm) -> exp-ACT (1114ns) -> ring
  slot release] with both engines running at ~95-100% of that budget.

  - QK^T per (pair, kb): the four K=64 stationaries (2 heads x 2 col-halves)
    land in 4 disjoint 64x64 PE quadrant groups (tile_position via base
    partitions); both 512-wide q-chunks of a quadrant are emitted adjacently
    so weight reloads hide under other quadrants' streams — up to 4-way
    concurrent matmul execution. Emitted under high_priority so ready score
    matmuls always preempt filler work.
  - exp: 56 tiles exact on ACT, 40 tiles (DVE_HEADS) as a one-instruction
    Schraudolph approximation on DVE (bf16 bits = s*128*log2e + const),
    draining each step's score pair in parallel (~9.5e-3 rel err total).
  - Softmax finalize on ACT: activation Copy with per-partition scale =
    reciprocal(denominator) from DVE — keeps DVE under the exp+cast load.
  - Projections: col-split M=64 halves ping-pong per contraction step (LDW
    hides under the opposite half's stream); PSUM->SBUF copies on DVE.
  - AV uses the exp tile as stationary and [V_h | 1] as 65-wide moving
    operand: col 64 accumulates the softmax denominator for free. All 8 vv
    blocks must exist before the first AV chain (av1 deferral).
  - x^T via two XBAR DMA transposes on sync (XBAR mis-executes on the scalar
    queue; copy-DMAs force a serializing xbar-mode transition, so W loads
    ride the scalar queue). Scratch matmuls warm the PE HAM clock gate.
  - One PSUM pool: 3x [128,1024] score tiles + 2x AV accumulators = 8 banks.

  Measured-and-rejected: ldweights=False reuse (walrus re-emits the load
  anyway), proj/AV high_priority (breaks the scheduler's filler placement),
  proj on the avp ring (PE fragmentation + HAM oscillation, 275us), a
  dedicated 2x1-bank proj ring with score ring depth 2 (zero jitter slack,
  207us; the ~66us of pair-boundary stalls did NOT clean up — they are not
  slot-assignment-driven), mid-kernel HAM filler matmuls, transposes-first
  DMA order, cast-to-ACT splits, av1 catch-up re-spreads, DVE_HEADS +head1.
  Run-to-run HW variance on this tunneled device is ~2-4us (one 232us
  outlier observed); v8 measured 197.3/197.5/198.8/201.0.
"""

import sys

sys.path.insert(0, "/opt/trn_rl_repo")

import numpy as np

B, S, D, H, DH = 8, 1024, 768, 12, 64
P = 128
ST = S // P  # 8 sequence tiles
DT = D // P  # 6 contraction tiles
NP = H // 2  # 6 head pairs (= n-tiles of 128)
QC = 512
N_CORES = 8
DVE_HEADS = frozenset({3, 5, 7, 9, 11})  # heads whose exp is DVE Schraudolph
N_WARMUP = 28

_STATE = {}


def _build():
    import concourse.mybir as mybir
    import concourse.tile as tile
    from concourse import bacc
    from contextlib import ExitStack

    f32 = mybir.dt.float32
    bf16 = mybir.dt.bfloat16
    i16 = mybir.dt.int16
    Exp = mybir.ActivationFunctionType.Exp
    Copy = mybir.ActivationFunctionType.Copy
    Alu = mybir.AluOpType

    nc = bacc.Bacc("TRN2", target_bir_lowering=False, debug=False)
    x_d = nc.dram_tensor("x", [S, D], bf16, kind="ExternalInput").ap()
    wq_d = nc.dram_tensor("WQ", [D, D], bf16, kind="ExternalInput").ap()
    wk_d = nc.dram_tensor("WK", [D, D], bf16, kind="ExternalInput").ap()
    wv_d = nc.dram_tensor("WV", [D, D], bf16, kind="ExternalInput").ap()
    out_d = nc.dram_tensor("out", [S, D], f32, kind="ExternalOutput").ap()

    with tile.TileContext(nc) as tc, ExitStack() as top:
        persist = top.enter_context(tc.tile_pool(name="persist", bufs=1))

        # warm the ACT exp table at t=0
        scr = persist.tile([1, 8], bf16)
        nc.vector.memset(scr[:], 0.0)
        nc.scalar.activation(scr[:], scr[:], Exp)

        qT = persist.tile([P, NP, S], bf16)
        kT = persist.tile([P, NP, S], bf16)
        vv = persist.tile([P, ST, H, DH + 1], bf16)  # V + ones col per head
        nc.vector.memset(vv[:, :, :, DH : DH + 1], 1.0)
        wrm = persist.tile([P, QC], bf16)
        nc.vector.memset(wrm[:], 0.0)

        xT = persist.tile([P, DT, S], bf16)
        # x^T via XBAR transposes on sync (the XBAR path mis-executes on the
        # scalar queue), pinned high-priority: Tile serializes every global
        # xbar-mode transition in SCHEDULED order, so if the scheduler floats
        # a W copy-DMA ahead of a transpose, the transpose waits for it.
        # Pinning the transposes first makes the copies wait instead (they
        # have slack; x^T is the critical path). W loads on the scalar queue.
        with tc.high_priority():
            nc.sync.dma_start_transpose(
                xT[:, 0:3, :], x_d[:, 0 : 3 * P]
            )
            nc.sync.dma_start_transpose(xT[:, 3:DT, :], x_d[:, 3 * P : D])

        with ExitStack() as s_w:
            wp = s_w.enter_context(tc.tile_pool(name="wp", bufs=1))
            wk = wp.tile([P, DT, D], bf16)
            wq = wp.tile([P, DT, D], bf16)
            wv = wp.tile([P, DT, D], bf16)
            wk_r = wk_d.rearrange("(d p) n -> p d n", p=P)
            wq_r = wq_d.rearrange("(d p) n -> p d n", p=P)
            nc.scalar.dma_start(wk[:, :, 0 : 2 * P], wk_r[:, :, 0 : 2 * P])
            nc.scalar.dma_start(wq[:, :, 0 : 2 * P], wq_r[:, :, 0 : 2 * P])
            nc.scalar.dma_start(
                wv[:, :, :], wv_d.rearrange("(d p) n -> p d n", p=P)
            )
            nc.scalar.dma_start(wq[:, :, 2 * P : D], wq_r[:, :, 2 * P : D])
            nc.scalar.dma_start(wk[:, :, 2 * P : D], wk_r[:, :, 2 * P : D])

            with ExitStack() as ph2:
                ps_pool = ph2.enter_context(
                    tc.tile_pool(name="ps", bufs=1, space="PSUM")
                )
                exp_pool = ph2.enter_context(tc.tile_pool(name="exp", bufs=36))
                stg_pool = ph2.enter_context(tc.tile_pool(name="stg", bufs=6))

                # HAM warm-up: keep the PE busy on scratch matmuls while the
                # inputs stream in, so the projections run at 2.4 GHz
                for w_i in range(N_WARMUP):
                    wps = ps_pool.tile([P, QC], f32, tag="avp", bufs=2)
                    nc.tensor.matmul(
                        wps[:], lhsT=wrm[:, 0:P], rhs=wrm[:], start=True, stop=True
                    )

                def mm_cs(out2, lhsT2, rhs, start, stop):
                    """col-split matmul: two concurrent M=64 halves (disjoint
                    partitions — safe on HW, sim's bank-granular group check
                    needs skip_group_check on the upper half)."""
                    nc.tensor.matmul(
                        out2[0:64], lhsT=lhsT2[:, 0:64], rhs=rhs,
                        start=start, stop=stop,
                    )
                    nc.tensor.matmul(
                        out2[64:P], lhsT=lhsT2[:, 64:P], rhs=rhs,
                        start=start, stop=stop, skip_group_check=True,
                    )

                def proj_kq(w_sb, dst, nt, qc, on_act=False):
                    ps = ps_pool.tile([P, S], f32, tag="sc", bufs=3)
                    for dt_ in range(DT):
                        mm_cs(
                            ps[:, 0:QC],
                            w_sb[:, dt_, nt * P : (nt + 1) * P],
                            xT[:, dt_, qc * QC : (qc + 1) * QC],
                            dt_ == 0,
                            dt_ == DT - 1,
                        )
                    dslice = dst[:, nt, qc * QC : (qc + 1) * QC]
                    if on_act:
                        nc.scalar.copy(dslice, ps[:, 0:QC])
                    else:
                        nc.vector.tensor_copy(dslice, ps[:, 0:QC])

                def proj_v(st):
                    psv = ps_pool.tile([P, S], f32, tag="sc", bufs=3)
                    for off, ln in ((0, 512), (512, 256)):
                        for dt_ in range(DT):
                            mm_cs(
                                psv[:, off : off + ln],
                                xT[:, dt_, st * P : (st + 1) * P],
                                wv[:, dt_, off : off + ln],
                                dt_ == 0,
                                dt_ == DT - 1,
                            )
                    nc.vector.tensor_copy(
                        vv[:, st, :, 0:DH],
                        psv[:, 0:D].rearrange("p (h d) -> p h d", h=H),
                    )

                exp_tiles = {}
                SCH_A = 184.6650390625
                SCH_B = 16250.996

                def qk_exp(p, kb):
                    """scores + exp for both heads of pair p at k-block kb.
                    The four K=64 stationaries (2 heads x 2 col-halves) land
                    in 4 disjoint 64x64 PE quadrants; both q-chunks of a
                    quadrant are adjacent so its reload hides under the other
                    quadrants' streams — up to 4-way concurrent. DVE_HEADS
                    take the Vector-engine Schraudolph exp."""
                    with tc.high_priority():
                        ps_a = ps_pool.tile([P, S], f32, tag="sc", bufs=3)
                        ps_b = ps_pool.tile([P, S], f32, tag="sc", bufs=3)
                        pss = (ps_a, ps_b)
                        for half in (0, 1):
                            lo, hi = half * DH, half * DH + DH
                            kchunk = kT[lo:hi, p, kb * P : (kb + 1) * P]
                            ps = pss[half]
                            for cl in (0, 1):
                                for qc in range(2):
                                    sl = slice(qc * QC, (qc + 1) * QC)
                                    nc.tensor.matmul(
                                        ps[cl * DH : cl * DH + DH, sl],
                                        lhsT=kchunk[:, cl * DH : cl * DH + DH],
                                        rhs=qT[lo:hi, p, sl],
                                        start=True, stop=True,
                                        skip_group_check=cl == 1,
                                    )
                        for half in (0, 1):
                            h = 2 * p + half
                            et = exp_pool.tile([P, S], bf16, tag="et")
                            if h in DVE_HEADS:
                                nc.vector.tensor_scalar(
                                    et.bitcast(i16)[:], pss[half][:], SCH_A,
                                    SCH_B, Alu.mult, Alu.add,
                                )
                            else:
                                nc.scalar.activation(et[:], pss[half][:], Exp)
                            exp_tiles[(h, kb)] = et

                rec_pool = ph2.enter_context(tc.tile_pool(name="rec", bufs=4))

                def av_head_st(h, st, stg2, half, tag="avp", bufs=2):
                    avp = ps_pool.tile([P, DH + 1], f32, tag=tag, bufs=bufs)
                    for kb2 in range(ST):
                        nc.tensor.matmul(
                            avp[:],
                            lhsT=exp_tiles[(h, kb2)][:, st * P : (st + 1) * P],
                            rhs=vv[:, kb2, h, :],
                            start=(kb2 == 0),
                            stop=(kb2 == ST - 1),
                        )
                    rec = rec_pool.tile([P, 1], f32, tag="rec")
                    nc.vector.reciprocal(rec[:], avp[:, DH : DH + 1])
                    nc.scalar.activation(
                        stg2[:, half * DH : (half + 1) * DH], avp[:, 0:DH],
                        Copy, scale=rec[:],
                    )

                def av_pair_st(pp, st):
                    stg2 = stg_pool.tile([P, 2 * DH], f32, tag="stg")
                    av_head_st(2 * pp, st, stg2, 0)
                    av_head_st(2 * pp + 1, st, stg2, 1)
                    nc.sync.dma_start(
                        out_d[st * P : (st + 1) * P, 2 * pp * DH : (2 * pp + 2) * DH],
                        stg2[:],
                    )

                # pair 0 + pair 1 projections up front; V spreads over
                # pair-0's slot (after WV lands) into pair-1's first two kbs.
                proj_kq(wk, kT, 0, 0)
                proj_kq(wq, qT, 0, 0)
                proj_kq(wq, qT, 0, 1)
                proj_kq(wk, kT, 0, 1)
                qk_exp(0, 0)
                for g in range(4):
                    w_sb, dst = ((wk, kT), (wq, qT))[g % 2]
                    proj_kq(w_sb, dst, 1, g // 2)
                for kb in range(1, ST):
                    qk_exp(0, kb)
                    if kb >= 2:
                        proj_v(kb - 2)

                # vv (all 8 st blocks) must be complete before the FIRST AV
                # chain — every chain contracts over all kb2; hence proj_v(6/7)
                # at kb 0/1 and the pair-0 AV catch-up deferred to kb >= 2.
                av1 = [(), (), (0, 1), (2, 3), (4,), (5,), (6,), (7,)]
                for p in range(1, NP):
                    for kb in range(ST):
                        qk_exp(p, kb)
                        if p < NP - 1 and kb < 4:
                            w_sb, dst = ((wk, kT), (wq, qT))[kb % 2]
                            proj_kq(w_sb, dst, p + 1, kb // 2)
                        if p == 1 and kb < 2:
                            proj_v(6 + kb)
                        for st in (av1[kb] if p == 1 else (kb,)):
                            av_pair_st(p - 1, st)
                    if p >= 2:
                        for kb2 in range(ST):
                            del exp_tiles[(2 * (p - 2), kb2)]
                            del exp_tiles[(2 * (p - 2) + 1, kb2)]

                # tail: last pair's AV (head A on the avp ring, head B on the
                # sc ring so more groups pipeline), stores split across the
                # now-idle sync + scalar queues
                pL = NP - 1
                for st in range(ST):
                    stg2 = stg_pool.tile([P, 2 * DH], f32, tag="stg")
                    av_head_st(2 * pL, st, stg2, 0)
                    av_head_st(2 * pL + 1, st, stg2, 1, tag="sc", bufs=3)
                    eng = nc.sync if st % 2 == 0 else nc.scalar
                    eng.dma_start(
                        out_d[st * P : (st + 1) * P, 2 * pL * DH : (2 * pL + 2) * DH],
                        stg2[:],
                    )

    nc.compile()
    return nc


def _to_bf16(a):
    import ml_dtypes

    return np.ascontiguousarray(
        np.asarray(a, dtype=np.float32).astype(ml_dtypes.bfloat16)
    )


def make_in_maps(x, WQ, WK, WV):
    """Host-side prep: bf16 inputs, 1/sqrt(dh)=2^-3 folded into WK (exact)."""
    x = np.asarray(x, dtype=np.float32)
    wq = _to_bf16(WQ)
    wk = _to_bf16(np.asarray(WK, dtype=np.float32) * np.float32(0.125))
    wv = _to_bf16(WV)
    return [
        {"x": _to_bf16(x[b]), "WQ": wq, "WK": wk, "WV": wv} for b in range(B)
    ]


def kernel(x, WQ, WK, WV):
    from concourse.bass_utils import run_bass_kernel_spmd

    assert np.asarray(x).shape == (B, S, D)
    if "nc" not in _STATE:
        _STATE["nc"] = _build()
    nc = _STATE["nc"]

    in_maps = make_in_maps(x, WQ, WK, WV)
    last_err = None
    for _ in range(3):  # retries: axon device errors are occasionally transient
        try:
            res = run_bass_kernel_spmd(nc, in_maps, list(range(N_CORES)))
            return np.stack([res.results[b]["out"] for b in range(B)], axis=0)
        except Exception as e:  # noqa: BLE001
            last_err = e
            import time

            time.sleep(3.0)
    raise last_err


if __name__ == "__main__":
    rng = np.random.default_rng(0)
    scale = 1.0 / np.float32(np.sqrt(D))
    ins = {
        "x": rng.standard_normal((B, S, D), dtype=np.float32),
        "WQ": rng.standard_normal((D, D), dtype=np.float32) * scale,
        "WK": rng.standard_normal((D, D), dtype=np.float32) * scale,
        "WV": rng.standard_normal((D, D), dtype=np.float32) * scale,
    }
    out = kernel(**ins)
    print(out.shape, out.dtype)

